# revision 12
# baseline (speedup 1.0000x reference)
"""ChannelWiseDivergence (nms_detection) — Trainium2 Bass kernel, 8 NeuronCores.

Pipeline:
  1. dice: per teacher proposal n: I=sum(x*t), X=sum(x^2), T=sum(t)
     over 192*192 pixels -> dice loss. Data-parallel: 80 of 640 rows/core.
  2. host: per-gt segmented argmin over the 640 dice losses (tiny).
  3. KL: per gt channel g: Zt=sum(exp(t)), Zs=sum(exp(s)),
     A=sum(exp(t)*t), B=sum(exp(t)*s); kl_g=(A-B)/Zt - log Zt + log Zs.
     Data-parallel: 16 of 128 channels/core.

Device layout: a [R, 36864] row-shard reshapes exactly to [R*8, 4608];
tiles of 128 partitions give per-partition reductions. The 8-partition
group sums are done on host (tiny [128, ncols] outputs).

Key implementation points vs the naive version:
  - All SBUF chunk buffers are persistent (no pool recycling), and every
    input dma_start is issued up front in arrival order. HWDGE DMAs drain
    FIFO per issuing engine, so chunks complete sequentially while the
    16 SDMA engines stay saturated.
  - Reductions use the DVE fused-accumulate ops (scalar_tensor_tensor /
    tensor_scalar with accum_out): one pass per stat, no fold chains.
  - First/last partition-tiles are column-split so compute starts early
    and the post-DMA tail is short.

Inputs are converted to bf16 on host (validated: identical argmin vs
f64; final KL rel err ~1e-6). All accumulation is fp32 on device.
"""

import numpy as np
import ml_dtypes

import concourse.tile as tile
from concourse import bacc, mybir
from concourse.bass_utils import run_bass_kernel_spmd

N_CORES = 8
N_T, G, HW = 640, 128, 192 * 192
R = N_T // N_CORES          # 80 teacher rows per core (phase 1)
CH = G // N_CORES           # 16 gt channels per core (phase 2)
E = HW // 8                 # 4608 = eighth-row length
Q1 = R * 8                  # 640 partition-rows per core, phase 1
NTILE1 = Q1 // 128          # 5 tiles of [128, 4608]
EPS = 1e-5

# phase-1 chunks: (tile_idx, col_lo, col_hi); tiles 0 and 4 are split so
# compute starts early and the tail after the last DMA is short.
P1_CHUNKS = [
    (0, 0, 2304), (0, 2304, 4608),
    (1, 0, 4608), (2, 0, 4608), (3, 0, 4608),
    (4, 0, 2304), (4, 2304, 4608),
]
NCH1 = len(P1_CHUNKS)

# phase-2 column chunks of the [128, 4608] per-core block
P2_CHUNKS = [(0, 1536), (1536, 3072), (3072, 4608)]
NCH2 = len(P2_CHUNKS)

BF16 = mybir.dt.bfloat16
F32 = mybir.dt.float32
_nb = ml_dtypes.bfloat16

_built = {}
LAST_RESULTS = {}


def _build_phase1():
    nc = bacc.Bacc("TRN2", target_bir_lowering=False, debug=False)
    x_in = nc.declare_dram_parameter("x", [Q1, E], BF16, isOutput=False)
    t_in = nc.declare_dram_parameter("t", [Q1, E], BF16, isOutput=False)
    # cols: I x NCH1 | X x NCH1 | T x NCH1
    stats = nc.declare_dram_parameter("stats", [128, 3 * NCH1], F32, isOutput=True)

    from contextlib import ExitStack
    with tile.TileContext(nc) as tc, ExitStack() as ctx:
        xp = ctx.enter_context(tc.tile_pool(name="xp", bufs=1))
        tp = ctx.enter_context(tc.tile_pool(name="tp", bufs=1))
        scr = ctx.enter_context(tc.tile_pool(name="scr", bufs=2))
        accp = ctx.enter_context(tc.tile_pool(name="acc", bufs=1))

        accs = accp.tile([128, 3 * NCH1], F32, tag="accs")

        # persistent chunk buffers; all loads issued up front in order
        xts, gts = [], []
        for ci, (it, lo, hi) in enumerate(P1_CHUNKS):
            w = hi - lo
            xt = xp.tile([128, w], BF16, tag=f"x{ci}", name=f"x{ci}")
            gt = tp.tile([128, w], BF16, tag=f"t{ci}", name=f"t{ci}")
            xts.append(xt)
            gts.append(gt)
        # Loads split across the two DGE paths so each stays within its
        # 8-semaphore in-flight budget (no issue gating): x + stats on
        # sync/HWDGE (8 DMAs), t on gpsimd/SWDGE (7 DMAs, issued by the
        # otherwise-idle Q7 so no compute engine pays issue cost).
        for ci, (it, lo, hi) in enumerate(P1_CHUNKS):
            sl = slice(it * 128, (it + 1) * 128)
            nc.sync.dma_start(out=xts[ci], in_=x_in[sl, lo:hi])
            nc.gpsimd.dma_start(out=gts[ci], in_=t_in[sl, lo:hi])

        # T-path engine split (balances DVE vs ACT busy time):
        # chunks 0-3 (= tiles 0ab, 1, 2) reduce on ACT; 4,5,6 on DVE.
        T_ON_ACT = (0, 1, 2, 3)
        for ci, (it, lo, hi) in enumerate(P1_CHUNKS):
            w = hi - lo
            xt, gt = xts[ci], gts[ci]
            tcol = accs[:, 2 * NCH1 + ci:2 * NCH1 + ci + 1]
            # I = sum(x*t): fused multiply+row-sum on DVE (1x rate, but one
            # pass — beats mul + fold chain by ~1us/tile)
            prod = scr.tile([128, w], BF16, tag="prod", name=f"prod{ci}")
            nc.vector.scalar_tensor_tensor(
                out=prod, in0=xt, scalar=1.0, in1=gt,
                op0=mybir.AluOpType.mult, op1=mybir.AluOpType.mult,
                accum_out=accs[:, ci:ci + 1],
            )
            # X = sum(x^2): ACT square with accumulate, element output
            # discarded into a stride-0 broadcast dummy
            sqx = scr.tile([128, 1], BF16, tag="sqx", name=f"sqx{ci}")
            nc.scalar.activation(
                out=sqx.broadcast_to(xt.shape), in_=xt,
                func=mybir.ActivationFunctionType.Square,
                accum_out=accs[:, NCH1 + ci:NCH1 + ci + 1],
            )
            # T = sum(t) (t is 0/1 so sum(t^2) == sum(t); host verifies)
            if ci in T_ON_ACT:
                sqt = scr.tile([128, 1], BF16, tag="sqt", name=f"sqt{ci}")
                nc.scalar.activation(
                    out=sqt.broadcast_to(gt.shape), in_=gt,
                    func=mybir.ActivationFunctionType.Square,
                    accum_out=tcol,
                )
            else:
                # fold at bf16 2x rate, then 1x-reduce only w/8 elems
                h1 = scr.tile([128, w // 2], BF16, tag="h1", name=f"h1_{ci}")
                nc.vector.tensor_add(h1, gt[:, :w // 2], gt[:, w // 2:])
                h2 = scr.tile([128, w // 4], BF16, tag="h2", name=f"h2_{ci}")
                nc.vector.tensor_add(h2, h1[:, :w // 4], h1[:, w // 4:])
                h3 = scr.tile([128, w // 8], BF16, tag="h3", name=f"h3_{ci}")
                nc.vector.tensor_add(h3, h2[:, :w // 8], h2[:, w // 8:])
                nc.vector.tensor_reduce(
                    out=tcol, in_=h3,
                    axis=mybir.AxisListType.X, op=mybir.AluOpType.add,
                )

        nc.sync.dma_start(out=stats[:, :], in_=accs)
    nc.finalize()
    return nc


def _build_phase2():
    nc = bacc.Bacc("TRN2", target_bir_lowering=False, debug=False)
    t_in = nc.declare_dram_parameter("t", [CH * 8, E], BF16, isOutput=False)
    s_in = nc.declare_dram_parameter("s", [CH * 8, E], BF16, isOutput=False)
    # cols: Zt x C | Zs x C | D x C   where D = sum(exp(t) * (t - s))
    stats = nc.declare_dram_parameter("stats2", [128, 3 * NCH2], F32, isOutput=True)

    from contextlib import ExitStack
    with tile.TileContext(nc) as tc, ExitStack() as ctx:
        tpool = ctx.enter_context(tc.tile_pool(name="tpool", bufs=1))
        spool = ctx.enter_context(tc.tile_pool(name="spool", bufs=1))
        scr = ctx.enter_context(tc.tile_pool(name="scr", bufs=2))
        accp = ctx.enter_context(tc.tile_pool(name="acc", bufs=1))

        acc = accp.tile([128, 3 * NCH2], F32, tag="acc")

        tts, sss = [], []
        for ci, (lo, hi) in enumerate(P2_CHUNKS):
            w = hi - lo
            tts.append(tpool.tile([128, w], BF16, tag=f"tt{ci}", name=f"tt{ci}"))
            sss.append(spool.tile([128, w], BF16, tag=f"ss{ci}", name=f"ss{ci}"))
        # dual DGE paths: t chunks on sync/HWDGE, s chunks on gpsimd/SWDGE
        for ci, (lo, hi) in enumerate(P2_CHUNKS):
            nc.sync.dma_start(out=tts[ci], in_=t_in[:, lo:hi])
        for ci, (lo, hi) in enumerate(P2_CHUNKS):
            nc.gpsimd.dma_start(out=sss[ci], in_=s_in[:, lo:hi])

        # preload the exp table set before any data arrives
        dum = scr.tile([128, 8], BF16, tag="dum")
        nc.vector.memset(dum, 0.0)
        nc.scalar.activation(
            out=dum, in_=dum, func=mybir.ActivationFunctionType.Exp,
        )

        ets = []
        for ci, (lo, hi) in enumerate(P2_CHUNKS):
            w = hi - lo
            et = scr.tile([128, w], BF16, tag="et", name=f"et{ci}")
            nc.scalar.activation(
                out=et, in_=tts[ci], func=mybir.ActivationFunctionType.Exp,
                accum_out=acc[:, ci:ci + 1],
            )
            ets.append(et)
        for ci, (lo, hi) in enumerate(P2_CHUNKS):
            w = hi - lo
            # D = sum(e^t * (t - s)): DVE sub at 2x, then fused mul+row-sum
            diff = scr.tile([128, w], BF16, tag="diff", name=f"diff{ci}")
            nc.vector.tensor_sub(diff, tts[ci], sss[ci])
            pd = scr.tile([128, w], BF16, tag="pd", name=f"pd{ci}")
            nc.vector.scalar_tensor_tensor(
                out=pd, in0=ets[ci], scalar=1.0, in1=diff,
                op0=mybir.AluOpType.mult, op1=mybir.AluOpType.mult,
                accum_out=acc[:, 2 * NCH2 + ci:2 * NCH2 + ci + 1],
            )
            # Zs = sum(e^s), element output discarded
            es = scr.tile([128, 1], BF16, tag="es", name=f"es{ci}")
            nc.scalar.activation(
                out=es.broadcast_to(sss[ci].shape), in_=sss[ci],
                func=mybir.ActivationFunctionType.Exp,
                accum_out=acc[:, NCH2 + ci:NCH2 + ci + 1],
            )

        nc.sync.dma_start(out=stats[:, :], in_=acc)
    nc.finalize()
    return nc


def _get(name, builder):
    if name not in _built:
        _built[name] = builder()
    return _built[name]


def kernel(preds_T, preds_S, im_ind, gt_T, gt_S, iter, gt_inds_T, gt_inds_S,
           **_unused):
    preds_T = np.asarray(preds_T, dtype=np.float32).reshape(N_T, HW)
    gt_T = np.asarray(gt_T, dtype=np.float32).reshape(N_T, HW)
    preds_S = np.asarray(preds_S, dtype=np.float32).reshape(G, HW)
    gt_inds_T = np.asarray(gt_inds_T).astype(np.int64)
    gt_inds_S = np.asarray(gt_inds_S).astype(np.int64)

    xb = preds_T.astype(_nb)
    tb = gt_T.astype(_nb)

    core_ids = list(range(N_CORES))

    # ---- phase 1: dice stats ----
    nc1 = _get("p1", _build_phase1)
    in_maps = []
    for i in core_ids:
        sl = slice(i * R, (i + 1) * R)
        in_maps.append({
            "x": np.ascontiguousarray(xb[sl]).reshape(Q1, E),
            "t": np.ascontiguousarray(tb[sl]).reshape(Q1, E),
        })
    res1 = run_bass_kernel_spmd(nc1, in_maps, core_ids)
    LAST_RESULTS["phase1"] = res1

    I = np.empty(N_T, np.float32)
    X = np.empty(N_T, np.float32)
    T = np.empty(N_T, np.float32)
    for i in core_ids:
        st = res1.results[i]["stats"]           # [128, 3*NCH1]
        per_q = np.zeros((3, Q1), np.float32)   # index q = it*128 + p
        for ci, (it, lo, hi) in enumerate(P1_CHUNKS):
            per_q[0, it * 128:(it + 1) * 128] += st[:, ci]
            per_q[1, it * 128:(it + 1) * 128] += st[:, NCH1 + ci]
            per_q[2, it * 128:(it + 1) * 128] += st[:, 2 * NCH1 + ci]
        # partition-row q = it*128 + p  ->  (local row r, eighth h) = divmod(q, 8)
        per_row = per_q.reshape(3, R, 8).sum(axis=2)
        I[i * R:(i + 1) * R] = per_row[0]
        X[i * R:(i + 1) * R] = per_row[1]
        T[i * R:(i + 1) * R] = per_row[2]

    # device computed T = sum(t), equal to sum(t^2) only for 0/1 gt
    if not bool((np.equal(gt_T, 0.0) | np.equal(gt_T, 1.0)).all()):
        tb32 = tb.astype(np.float32)
        T = (tb32 * tb32).sum(axis=1)

    loss = 1.0 - 2.0 * I / (X + T + np.float32(EPS))

    # segmented argmin with first-index tie-break (matches reference)
    seg_min = np.full(G, np.inf, np.float32)
    np.minimum.at(seg_min, gt_inds_T, loss)
    cand = np.where(loss == seg_min[gt_inds_T], np.arange(N_T), N_T)
    nms_inds = np.full(G, N_T, np.int64)
    np.minimum.at(nms_inds, gt_inds_T, cand)

    # match(): channel_T = preds_T[nms_inds][gt_inds_S]
    ch_T = xb[nms_inds[gt_inds_S]]              # [G, HW] bf16
    ch_S = preds_S.astype(_nb)                  # [G, HW] bf16

    # ---- phase 2: KL stats ----
    nc2 = _get("p2", _build_phase2)
    in_maps2 = []
    for i in core_ids:
        sl = slice(i * CH, (i + 1) * CH)
        in_maps2.append({
            "t": np.ascontiguousarray(ch_T[sl]).reshape(CH * 8, E),
            "s": np.ascontiguousarray(ch_S[sl]).reshape(CH * 8, E),
        })
    res2 = run_bass_kernel_spmd(nc2, in_maps2, core_ids)
    LAST_RESULTS["phase2"] = res2

    kl = 0.0
    for i in core_ids:
        st = res2.results[i]["stats2"].astype(np.float64)    # [128, 3*NCH2]
        per_p = st.reshape(128, 3, NCH2).sum(axis=2)         # [128, (Zt,Zs,D)]
        zt, zs, dd = per_p.reshape(CH, 8, 3).sum(axis=1).T   # each [CH]
        kl += (dd / zt - np.log(zt) + np.log(zs)).sum()

    return np.asarray(kl, dtype=np.float32)


# revision 18
# speedup vs baseline: 1.0691x; 1.0691x over previous
"""ChannelWiseDivergence (nms_detection) — Trainium2 Bass kernel, 8 NeuronCores.

Pipeline:
  1. dice: per teacher proposal n: I=sum(x*t), X=sum(x^2), T=sum(t)
     over 192*192 pixels -> dice loss. Data-parallel: 80 of 640 rows/core.
  2. host: per-gt segmented argmin over the 640 dice losses (tiny).
  3. KL: per gt channel g: Zt=sum(exp(t)), Zs=sum(exp(s)),
     A=sum(exp(t)*t), B=sum(exp(t)*s); kl_g=(A-B)/Zt - log Zt + log Zs.
     Data-parallel: 16 of 128 channels/core.

Device layout: a [R, 36864] row-shard reshapes exactly to [R*8, 4608];
tiles of 128 partitions give per-partition reductions. The 8-partition
group sums are done on host (tiny [128, ncols] outputs).

Key implementation points vs the naive version:
  - All SBUF chunk buffers are persistent (no pool recycling), and every
    input dma_start is issued up front in arrival order. HWDGE DMAs drain
    FIFO per issuing engine, so chunks complete sequentially while the
    16 SDMA engines stay saturated.
  - Reductions use the DVE fused-accumulate ops (scalar_tensor_tensor /
    tensor_scalar with accum_out): one pass per stat, no fold chains.
  - First/last partition-tiles are column-split so compute starts early
    and the post-DMA tail is short.

Inputs are converted to bf16 on host (validated: identical argmin vs
f64; final KL rel err ~1e-6). All accumulation is fp32 on device.
"""

import numpy as np
import ml_dtypes

import concourse.tile as tile
from concourse import bacc, mybir
from concourse.bass_utils import run_bass_kernel_spmd

N_CORES = 8
N_T, G, HW = 640, 128, 192 * 192
R = N_T // N_CORES          # 80 teacher rows per core (phase 1)
CH = G // N_CORES           # 16 gt channels per core (phase 2)
E = HW // 8                 # 4608 = eighth-row length
Q1 = R * 8                  # 640 partition-rows per core, phase 1
NTILE1 = Q1 // 128          # 5 tiles of [128, 4608]
EPS = 1e-5

# phase-1 chunks: (tile_idx, col_lo, col_hi); tiles 0 and 4 are split so
# compute starts early and the tail after the last DMA is short.
P1_CHUNKS = [
    (0, 0, 2304), (0, 2304, 4608),
    (1, 0, 4608), (2, 0, 4608), (3, 0, 4608),
    (4, 0, 2304), (4, 2304, 4608),
]
NCH1 = len(P1_CHUNKS)

# phase-2 column chunks of the [128, 4608] per-core block
P2_CHUNKS = [(0, 1536), (1536, 3072), (3072, 4608)]
NCH2 = len(P2_CHUNKS)

BF16 = mybir.dt.bfloat16
F32 = mybir.dt.float32
_nb = ml_dtypes.bfloat16

_built = {}
LAST_RESULTS = {}


def _build_phase1():
    nc = bacc.Bacc("TRN2", target_bir_lowering=False, debug=False)
    x_in = nc.declare_dram_parameter("x", [Q1, E], BF16, isOutput=False)
    t_in = nc.declare_dram_parameter("t", [Q1, E], BF16, isOutput=False)
    # cols: I x NCH1 | X x NCH1 | T x NCH1
    stats = nc.declare_dram_parameter("stats", [128, 3 * NCH1], F32, isOutput=True)

    from contextlib import ExitStack
    with tile.TileContext(nc) as tc, ExitStack() as ctx:
        xp = ctx.enter_context(tc.tile_pool(name="xp", bufs=1))
        tp = ctx.enter_context(tc.tile_pool(name="tp", bufs=1))
        scr = ctx.enter_context(tc.tile_pool(name="scr", bufs=2))
        accp = ctx.enter_context(tc.tile_pool(name="acc", bufs=1))

        accs = accp.tile([128, 3 * NCH1], F32, tag="accs")

        # persistent chunk buffers; all loads issued up front in order
        xts, gts = [], []
        for ci, (it, lo, hi) in enumerate(P1_CHUNKS):
            w = hi - lo
            xt = xp.tile([128, w], BF16, tag=f"x{ci}", name=f"x{ci}")
            gt = tp.tile([128, w], BF16, tag=f"t{ci}", name=f"t{ci}")
            xts.append(xt)
            gts.append(gt)
        # All loads issued up front on the sync/HWDGE ring in arrival
        # order; they drain near-FIFO so chunk pairs complete sequentially.
        for ci, (it, lo, hi) in enumerate(P1_CHUNKS):
            sl = slice(it * 128, (it + 1) * 128)
            nc.sync.dma_start(out=xts[ci], in_=x_in[sl, lo:hi])
            nc.sync.dma_start(out=gts[ci], in_=t_in[sl, lo:hi])

        # T-path engine split (balances DVE vs ACT busy time):
        # chunks 0,1,2 (= tiles 0ab, 1) reduce on ACT; 3,4,5,6 on DVE.
        T_ON_ACT = (0, 1, 2)
        for ci, (it, lo, hi) in enumerate(P1_CHUNKS):
            w = hi - lo
            xt, gt = xts[ci], gts[ci]
            tcol = accs[:, 2 * NCH1 + ci:2 * NCH1 + ci + 1]
            # I = sum(x*t): fused multiply+row-sum on DVE (1x rate, but one
            # pass — beats mul + fold chain by ~1us/tile)
            prod = scr.tile([128, w], BF16, tag="prod", name=f"prod{ci}")
            nc.vector.scalar_tensor_tensor(
                out=prod, in0=xt, scalar=1.0, in1=gt,
                op0=mybir.AluOpType.mult, op1=mybir.AluOpType.mult,
                accum_out=accs[:, ci:ci + 1],
            )
            # X = sum(x^2): ACT square with accumulate, element output
            # discarded into a stride-0 broadcast dummy
            sqx = scr.tile([128, 1], BF16, tag="sqx", name=f"sqx{ci}")
            nc.scalar.activation(
                out=sqx.broadcast_to(xt.shape), in_=xt,
                func=mybir.ActivationFunctionType.Square,
                accum_out=accs[:, NCH1 + ci:NCH1 + ci + 1],
            )
            # T = sum(t) (t is 0/1 so sum(t^2) == sum(t); host verifies)
            if ci in T_ON_ACT:
                sqt = scr.tile([128, 1], BF16, tag="sqt", name=f"sqt{ci}")
                nc.scalar.activation(
                    out=sqt.broadcast_to(gt.shape), in_=gt,
                    func=mybir.ActivationFunctionType.Square,
                    accum_out=tcol,
                )
            else:
                # fold at bf16 2x rate, then 1x-reduce only w/8 elems
                h1 = scr.tile([128, w // 2], BF16, tag="h1", name=f"h1_{ci}")
                nc.vector.tensor_add(h1, gt[:, :w // 2], gt[:, w // 2:])
                h2 = scr.tile([128, w // 4], BF16, tag="h2", name=f"h2_{ci}")
                nc.vector.tensor_add(h2, h1[:, :w // 4], h1[:, w // 4:])
                h3 = scr.tile([128, w // 8], BF16, tag="h3", name=f"h3_{ci}")
                nc.vector.tensor_add(h3, h2[:, :w // 8], h2[:, w // 8:])
                nc.vector.tensor_reduce(
                    out=tcol, in_=h3,
                    axis=mybir.AxisListType.X, op=mybir.AluOpType.add,
                )

        nc.sync.dma_start(out=stats[:, :], in_=accs)
    nc.finalize()
    return nc


def _build_phase2():
    nc = bacc.Bacc("TRN2", target_bir_lowering=False, debug=False)
    t_in = nc.declare_dram_parameter("t", [CH * 8, E], BF16, isOutput=False)
    s_in = nc.declare_dram_parameter("s", [CH * 8, E], BF16, isOutput=False)
    # cols: Zt x C | Zs x C | D x C   where D = sum(exp(t) * (t - s))
    stats = nc.declare_dram_parameter("stats2", [128, 3 * NCH2], F32, isOutput=True)

    from contextlib import ExitStack
    with tile.TileContext(nc) as tc, ExitStack() as ctx:
        tpool = ctx.enter_context(tc.tile_pool(name="tpool", bufs=1))
        spool = ctx.enter_context(tc.tile_pool(name="spool", bufs=1))
        scr = ctx.enter_context(tc.tile_pool(name="scr", bufs=2))
        accp = ctx.enter_context(tc.tile_pool(name="acc", bufs=1))

        acc = accp.tile([128, 3 * NCH2], F32, tag="acc")
        nc.vector.memset(acc, 0.0)

        tts = []
        for ci, (lo, hi) in enumerate(P2_CHUNKS):
            w = hi - lo
            tts.append(tpool.tile([128, w], BF16, tag=f"tt{ci}", name=f"tt{ci}"))
        ss_all = spool.tile([128, E], BF16, tag="ss_all")
        # interleave t/s chunk loads so each chunk pair lands together
        for ci, (lo, hi) in enumerate(P2_CHUNKS):
            nc.sync.dma_start(out=tts[ci], in_=t_in[:, lo:hi])
            nc.sync.dma_start(out=ss_all[:, lo:hi], in_=s_in[:, lo:hi])

        # preload the exp table set before any data arrives
        dum = scr.tile([128, 8], BF16, tag="dum")
        nc.vector.memset(dum, 0.0)
        nc.scalar.activation(
            out=dum, in_=dum, func=mybir.ActivationFunctionType.Exp,
        )

        ets = []
        for ci, (lo, hi) in enumerate(P2_CHUNKS):
            w = hi - lo
            et = scr.tile([128, w], BF16, tag="et", name=f"et{ci}")
            nc.scalar.activation(
                out=et, in_=tts[ci], func=mybir.ActivationFunctionType.Exp,
                accum_out=acc[:, ci:ci + 1],
            )
            ets.append(et)
        for ci, (lo, hi) in enumerate(P2_CHUNKS):
            w = hi - lo
            # D = sum(e^t * (t - s)): DVE sub at 2x, then fused mul+row-sum
            diff = scr.tile([128, w], BF16, tag="diff", name=f"diff{ci}")
            nc.vector.tensor_sub(diff, tts[ci], ss_all[:, lo:hi])
            pd = scr.tile([128, w], BF16, tag="pd", name=f"pd{ci}")
            nc.vector.scalar_tensor_tensor(
                out=pd, in0=ets[ci], scalar=1.0, in1=diff,
                op0=mybir.AluOpType.mult, op1=mybir.AluOpType.mult,
                accum_out=acc[:, 2 * NCH2 + ci:2 * NCH2 + ci + 1],
            )
        # Zs = sum(e^s) in one pass over the contiguous s block
        es = scr.tile([128, 1], BF16, tag="es")
        nc.scalar.activation(
            out=es.broadcast_to(ss_all.shape), in_=ss_all,
            func=mybir.ActivationFunctionType.Exp,
            accum_out=acc[:, NCH2:NCH2 + 1],
        )

        nc.sync.dma_start(out=stats[:, :], in_=acc)
    nc.finalize()
    return nc


def _get(name, builder):
    if name not in _built:
        _built[name] = builder()
    return _built[name]


def kernel(preds_T, preds_S, im_ind, gt_T, gt_S, iter, gt_inds_T, gt_inds_S,
           **_unused):
    preds_T = np.asarray(preds_T, dtype=np.float32).reshape(N_T, HW)
    gt_T = np.asarray(gt_T, dtype=np.float32).reshape(N_T, HW)
    preds_S = np.asarray(preds_S, dtype=np.float32).reshape(G, HW)
    gt_inds_T = np.asarray(gt_inds_T).astype(np.int64)
    gt_inds_S = np.asarray(gt_inds_S).astype(np.int64)

    xb = preds_T.astype(_nb)
    tb = gt_T.astype(_nb)

    core_ids = list(range(N_CORES))

    # ---- phase 1: dice stats ----
    nc1 = _get("p1", _build_phase1)
    in_maps = []
    for i in core_ids:
        sl = slice(i * R, (i + 1) * R)
        in_maps.append({
            "x": np.ascontiguousarray(xb[sl]).reshape(Q1, E),
            "t": np.ascontiguousarray(tb[sl]).reshape(Q1, E),
        })
    res1 = run_bass_kernel_spmd(nc1, in_maps, core_ids)
    LAST_RESULTS["phase1"] = res1

    I = np.empty(N_T, np.float32)
    X = np.empty(N_T, np.float32)
    T = np.empty(N_T, np.float32)
    for i in core_ids:
        st = res1.results[i]["stats"]           # [128, 3*NCH1]
        per_q = np.zeros((3, Q1), np.float32)   # index q = it*128 + p
        for ci, (it, lo, hi) in enumerate(P1_CHUNKS):
            per_q[0, it * 128:(it + 1) * 128] += st[:, ci]
            per_q[1, it * 128:(it + 1) * 128] += st[:, NCH1 + ci]
            per_q[2, it * 128:(it + 1) * 128] += st[:, 2 * NCH1 + ci]
        # partition-row q = it*128 + p  ->  (local row r, eighth h) = divmod(q, 8)
        per_row = per_q.reshape(3, R, 8).sum(axis=2)
        I[i * R:(i + 1) * R] = per_row[0]
        X[i * R:(i + 1) * R] = per_row[1]
        T[i * R:(i + 1) * R] = per_row[2]

    # device computed T = sum(t), equal to sum(t^2) only for 0/1 gt
    if not bool((np.equal(gt_T, 0.0) | np.equal(gt_T, 1.0)).all()):
        tb32 = tb.astype(np.float32)
        T = (tb32 * tb32).sum(axis=1)

    loss = 1.0 - 2.0 * I / (X + T + np.float32(EPS))

    # segmented argmin with first-index tie-break (matches reference)
    seg_min = np.full(G, np.inf, np.float32)
    np.minimum.at(seg_min, gt_inds_T, loss)
    cand = np.where(loss == seg_min[gt_inds_T], np.arange(N_T), N_T)
    nms_inds = np.full(G, N_T, np.int64)
    np.minimum.at(nms_inds, gt_inds_T, cand)

    # match(): channel_T = preds_T[nms_inds][gt_inds_S]
    ch_T = xb[nms_inds[gt_inds_S]]              # [G, HW] bf16
    ch_S = preds_S.astype(_nb)                  # [G, HW] bf16

    # ---- phase 2: KL stats ----
    nc2 = _get("p2", _build_phase2)
    in_maps2 = []
    for i in core_ids:
        sl = slice(i * CH, (i + 1) * CH)
        in_maps2.append({
            "t": np.ascontiguousarray(ch_T[sl]).reshape(CH * 8, E),
            "s": np.ascontiguousarray(ch_S[sl]).reshape(CH * 8, E),
        })
    res2 = run_bass_kernel_spmd(nc2, in_maps2, core_ids)
    LAST_RESULTS["phase2"] = res2

    kl = 0.0
    for i in core_ids:
        st = res2.results[i]["stats2"].astype(np.float64)    # [128, 3*NCH2]
        zt_p = st[:, 0:NCH2].sum(axis=1)                     # [128]
        zs_p = st[:, NCH2]                                   # single col
        dd_p = st[:, 2 * NCH2:3 * NCH2].sum(axis=1)
        zt = zt_p.reshape(CH, 8).sum(axis=1)
        zs = zs_p.reshape(CH, 8).sum(axis=1)
        dd = dd_p.reshape(CH, 8).sum(axis=1)
        kl += (dd / zt - np.log(zt) + np.log(zs)).sum()

    return np.asarray(kl, dtype=np.float32)


# revision 22
# speedup vs baseline: 1.1504x; 1.0760x over previous
"""ChannelWiseDivergence (nms_detection) — Trainium2 Bass kernel, 8 NeuronCores.

Pipeline:
  1. dice: per teacher proposal n: I=sum(x*t), X=sum(x^2), T=sum(t^2)
     over 192*192 pixels -> dice loss. Data-parallel: 80 of 640 rows/core.
  2. host: per-gt segmented argmin over the 640 dice losses (tiny).
  3. KL: per gt channel g: Z_t=sum(exp(t)), Z_s=sum(exp(s)),
     A=sum(exp(t)*t), B=sum(exp(t)*s); kl_g=(A-B)/Z_t - log Z_t + log Z_s.
     Data-parallel: 16 of 128 channels/core. (max-subtraction skipped:
     |logits| <= ~5.5, exp is safe in fp32.)

Device layout trick: a [R, 36864] row-shard reshapes exactly to
[R*8, 4608]; tiles of 128 partitions then give per-partition reductions
(DVE mul + fold + tensor_reduce, ACT activation with accum_out), and the
8-partition group sums are done on host (tiny [128,5] outputs).
(tensor_tensor_reduce would fuse mul+reduce but wedges real silicon.)

Inputs are converted to bf16 on host (validated: identical argmin vs
f64; final KL rel err ~5e-6). All accumulation is fp32 on device.
"""

import numpy as np
import ml_dtypes

import concourse.tile as tile
from concourse import bacc, mybir
from concourse.bass_utils import run_bass_kernel_spmd

N_CORES = 8
N_T, G, HW = 640, 128, 192 * 192
R = N_T // N_CORES          # 80 teacher rows per core (phase 1)
CH = G // N_CORES           # 16 gt channels per core (phase 2)
E = HW // 8                 # 4608 = eighth-row length
Q1 = R * 8                  # 640 partition-rows per core, phase 1
NTILE1 = Q1 // 128          # 5 tiles of [128, 4608]
P2C = 3                     # phase-2 column chunks (pipeline DMA/ACT/DVE)
EPS = 1e-5

BF16 = mybir.dt.bfloat16
F32 = mybir.dt.float32
_nb = ml_dtypes.bfloat16

_built = {}
LAST_RESULTS = {}


def _build_phase1():
    nc = bacc.Bacc("TRN2", target_bir_lowering=False, debug=False)
    x_in = nc.declare_dram_parameter("x", [Q1, E], BF16, isOutput=False)
    t_in = nc.declare_dram_parameter("t", [Q1, E], BF16, isOutput=False)
    stats = nc.declare_dram_parameter("stats", [128, 3 * NTILE1 + 1], F32, isOutput=True)

    from contextlib import ExitStack
    with tile.TileContext(nc) as tc, ExitStack() as ctx:
        io = ctx.enter_context(tc.tile_pool(name="io", bufs=4))
        scr = ctx.enter_context(tc.tile_pool(name="scr", bufs=3))
        accp = ctx.enter_context(tc.tile_pool(name="acc", bufs=1))

        accs = accp.tile([128, 3 * NTILE1 + 1], F32, tag="accs")
        iacc, xacc, tacc = (accs[:, 0:NTILE1], accs[:, NTILE1:2 * NTILE1],
                            accs[:, 2 * NTILE1:3 * NTILE1])

        for it in range(NTILE1):
            xt = io.tile([128, E], BF16, tag="xt")
            if it == 0:
                # split tile0's x-load + square in half: ACT's critical
                # chain starts ~1.5us earlier (half the DMA latency)
                nc.sync.dma_start(out=xt[:, :E // 2],
                                  in_=x_in[:128, :E // 2])
                nc.sync.dma_start(out=xt[:, E // 2:],
                                  in_=x_in[:128, E // 2:])
            else:
                nc.sync.dma_start(out=xt, in_=x_in[it * 128:(it + 1) * 128, :])
            gt = io.tile([128, E], BF16, tag="gt")
            nc.sync.dma_start(out=gt, in_=t_in[it * 128:(it + 1) * 128, :])

            # fused multiply + per-partition row-sum in ONE DVE pass
            # (InstTensorScalarPtr scalar_tensor_tensor with accum_out:
            # 4958ns/tile vs 6025ns for mul+fold+fold+reduce, HW-validated)
            prod = scr.tile([128, E], BF16, tag="prod")
            nc.vector.scalar_tensor_tensor(
                out=prod, in0=xt, scalar=1.0, in1=gt,
                op0=mybir.AluOpType.mult, op1=mybir.AluOpType.mult,
                accum_out=iacc[:, it:it + 1],
            )
            # squares' element outputs are discarded — write them to a
            # stride-0 broadcast dummy to save SBUF scratch + write BW
            sqx = scr.tile([128, 1], BF16, tag="sqx")
            if it == 0:
                nc.scalar.activation(
                    out=sqx.broadcast_to([128, E // 2]), in_=xt[:, :E // 2],
                    func=mybir.ActivationFunctionType.Square,
                    accum_out=xacc[:, 0:1],
                )
                sqx2 = scr.tile([128, 1], BF16, tag="sqx2")
                nc.scalar.activation(
                    out=sqx2.broadcast_to([128, E // 2]), in_=xt[:, E // 2:],
                    func=mybir.ActivationFunctionType.Square,
                    accum_out=accs[:, 3 * NTILE1:3 * NTILE1 + 1],
                )
            else:
                nc.scalar.activation(
                    out=sqx.broadcast_to(xt.shape), in_=xt,
                    func=mybir.ActivationFunctionType.Square,
                    accum_out=xacc[:, it:it + 1],
                )
            if it < 3:
                sqt = scr.tile([128, 1], BF16, tag="sqt")
                nc.scalar.activation(
                    out=sqt.broadcast_to(gt.shape), in_=gt,
                    func=mybir.ActivationFunctionType.Square,
                    accum_out=tacc[:, it:it + 1],
                )
            else:
                # sum(t) == sum(t^2) for 0/1 gt; host verifies + falls back
                halft = scr.tile([128, E // 2], BF16, tag="halft")
                nc.vector.tensor_add(halft, gt[:, :E // 2], gt[:, E // 2:])
                quartt = scr.tile([128, E // 4], BF16, tag="quartt")
                nc.vector.tensor_add(quartt, halft[:, :E // 4], halft[:, E // 4:])
                eightt = scr.tile([128, E // 8], BF16, tag="eightt")
                nc.vector.tensor_add(eightt, quartt[:, :E // 8], quartt[:, E // 8:])
                nc.vector.tensor_reduce(
                    out=tacc[:, it:it + 1], in_=eightt,
                    axis=mybir.AxisListType.X, op=mybir.AluOpType.add,
                )

        nc.sync.dma_start(out=stats[:, :], in_=accs)
    nc.finalize()
    return nc


def _build_phase2():
    nc = bacc.Bacc("TRN2", target_bir_lowering=False, debug=False)
    t_in = nc.declare_dram_parameter("t", [CH * 8, E], BF16, isOutput=False)
    s_in = nc.declare_dram_parameter("s", [CH * 8, E], BF16, isOutput=False)
    # cols: [Zt x C | Zs x C | D x C] where D = sum(exp(t) * (t - s))
    stats = nc.declare_dram_parameter("stats2", [128, 3 * P2C], F32, isOutput=True)

    from contextlib import ExitStack
    with tile.TileContext(nc) as tc, ExitStack() as ctx:
        io = ctx.enter_context(tc.tile_pool(name="io", bufs=4))
        scr = ctx.enter_context(tc.tile_pool(name="scr", bufs=3))
        accp = ctx.enter_context(tc.tile_pool(name="acc", bufs=1))

        acc = accp.tile([128, 3 * P2C], F32, tag="acc")
        CK = E // P2C
        for c in range(P2C):
            sl = slice(c * CK, (c + 1) * CK)
            tt = io.tile([128, CK], BF16, tag="tt")
            nc.sync.dma_start(out=tt, in_=t_in[:, sl])
            ss = io.tile([128, CK], BF16, tag="ss")
            nc.sync.dma_start(out=ss, in_=s_in[:, sl])

            et = scr.tile([128, CK], BF16, tag="et")
            nc.scalar.activation(
                out=et, in_=tt, func=mybir.ActivationFunctionType.Exp,
                accum_out=acc[:, c:c + 1],
            )
            dd = scr.tile([128, CK], BF16, tag="dd")
            nc.vector.tensor_sub(dd, tt, ss)
            es = scr.tile([128, 1], BF16, tag="es")
            nc.scalar.activation(
                out=es.broadcast_to(ss.shape), in_=ss,
                func=mybir.ActivationFunctionType.Exp,
                accum_out=acc[:, P2C + c:P2C + c + 1],
            )
            # fused multiply + row-sum: D_c = sum(e^t * (t-s)) in one pass
            pd = scr.tile([128, CK], BF16, tag="pd")
            nc.vector.scalar_tensor_tensor(
                out=pd, in0=et, scalar=1.0, in1=dd,
                op0=mybir.AluOpType.mult, op1=mybir.AluOpType.mult,
                accum_out=acc[:, 2 * P2C + c:2 * P2C + c + 1],
            )

        nc.sync.dma_start(out=stats[:, :], in_=acc)
    nc.finalize()
    return nc


def _get(name, builder):
    if name not in _built:
        _built[name] = builder()
    return _built[name]


def kernel(preds_T, preds_S, im_ind, gt_T, gt_S, iter, gt_inds_T, gt_inds_S,
           **_unused):
    preds_T = np.asarray(preds_T, dtype=np.float32).reshape(N_T, HW)
    gt_T = np.asarray(gt_T, dtype=np.float32).reshape(N_T, HW)
    preds_S = np.asarray(preds_S, dtype=np.float32).reshape(G, HW)
    gt_inds_T = np.asarray(gt_inds_T).astype(np.int64)
    gt_inds_S = np.asarray(gt_inds_S).astype(np.int64)

    xb = preds_T.astype(_nb)
    tb = gt_T.astype(_nb)

    core_ids = list(range(N_CORES))

    # ---- phase 1: dice stats ----
    nc1 = _get("p1", _build_phase1)
    in_maps = []
    for i in core_ids:
        sl = slice(i * R, (i + 1) * R)
        in_maps.append({
            "x": np.ascontiguousarray(xb[sl]).reshape(Q1, E),
            "t": np.ascontiguousarray(tb[sl]).reshape(Q1, E),
        })
    res1 = run_bass_kernel_spmd(nc1, in_maps, core_ids)
    LAST_RESULTS["phase1"] = res1

    I = np.empty(N_T, np.float32)
    X = np.empty(N_T, np.float32)
    T = np.empty(N_T, np.float32)
    for i in core_ids:
        st = res1.results[i]["stats"]           # [128, 3*NTILE1+1]
        # col 3*NTILE1 holds the second half of X tile0's split square
        st = st.copy()
        st[:, NTILE1] += st[:, 3 * NTILE1]
        st = st[:, :3 * NTILE1]
        # partition-row q = it*128 + p  ->  (local row r, eighth h) = divmod(q, 8)
        per_q = st.T.reshape(3, NTILE1, 128).reshape(3, Q1)   # index q
        per_row = per_q.reshape(3, R, 8).sum(axis=2)
        I[i * R:(i + 1) * R] = per_row[0]
        X[i * R:(i + 1) * R] = per_row[1]
        T[i * R:(i + 1) * R] = per_row[2]

    # 2 of 5 T-tiles summed t (not t^2) on device — exact only for 0/1 gt
    if not bool((np.equal(gt_T, 0.0) | np.equal(gt_T, 1.0)).all()):
        tb32 = tb.astype(np.float32)
        T = (tb32 * tb32).sum(axis=1)

    loss = 1.0 - 2.0 * I / (X + T + np.float32(EPS))

    # segmented argmin with first-index tie-break (matches reference)
    seg_min = np.full(G, np.inf, np.float32)
    np.minimum.at(seg_min, gt_inds_T, loss)
    cand = np.where(loss == seg_min[gt_inds_T], np.arange(N_T), N_T)
    nms_inds = np.full(G, N_T, np.int64)
    np.minimum.at(nms_inds, gt_inds_T, cand)

    # match(): channel_T = preds_T[nms_inds][gt_inds_S]
    ch_T = xb[nms_inds[gt_inds_S]]              # [G, HW] bf16
    ch_S = preds_S.astype(_nb)                  # [G, HW] bf16

    # ---- phase 2: KL stats ----
    nc2 = _get("p2", _build_phase2)
    in_maps2 = []
    for i in core_ids:
        sl = slice(i * CH, (i + 1) * CH)
        in_maps2.append({
            "t": np.ascontiguousarray(ch_T[sl]).reshape(CH * 8, E),
            "s": np.ascontiguousarray(ch_S[sl]).reshape(CH * 8, E),
        })
    res2 = run_bass_kernel_spmd(nc2, in_maps2, core_ids)
    LAST_RESULTS["phase2"] = res2

    kl = 0.0
    for i in core_ids:
        st = res2.results[i]["stats2"].astype(np.float64)    # [128, 3*P2C]
        per_p = st.reshape(128, 3, P2C).sum(axis=2)          # [128, (Zt,Zs,D)]
        zt, zs, dd = per_p.reshape(CH, 8, 3).sum(axis=1).T   # each [CH]
        kl += (dd / zt - np.log(zt) + np.log(zs)).sum()

    return np.asarray(kl, dtype=np.float32)

